# revision 1
# baseline (speedup 1.0000x reference)
"""DSSIM loss kernel for Trainium2, 8 NeuronCores, data-parallel over batch.

Math: for each (b, c) 512x512 image pair (x, y):
  s = x + y, d = x - y
  S = conv(s), D = conv(d), P = conv(s^2), Q = conv(d^2)   (separable 11-tap gaussian)
  2*mu1*mu2      = (S^2 - D^2)/2
  mu1^2 + mu2^2  = (S^2 + D^2)/2
  2*sigma12 + C2       = (P - Q)/2 + C2 - (S^2 - D^2)/2
  sigma1+sigma2 + C2   = (P + Q)/2 + C2 - (S^2 + D^2)/2
  ssim = ((2mu1mu2 + C1) * (2sigma12 + C2)) / ((mu1^2+mu2^2+C1) * (sigma1+sigma2+C2))
  DSSIM = 1 - mean(ssim)

Each separable conv = two banded-matrix multiplies on the PE:
  pass1 (image as stationary operand) convolves H and transposes;
  pass2 (gaussian band as stationary) convolves W via overlap-save 118-row chunks.
P-Q and P+Q are formed directly in PSUM with +/-G weights in pass2.
Per-core output: per-partition running sums of the ssim map; host reduces.
"""

import numpy as np
import ml_dtypes

import concourse.bass as bass
import concourse.bacc as bacc
import concourse.tile as tile
from concourse import mybir
from concourse.bass_utils import run_bass_kernel_spmd

AOP = mybir.AluOpType
ACTF = mybir.ActivationFunctionType

# problem constants (hardcoded per harness contract)
FULL_B, CH, H, W = 16, 3, 512, 512
N_CORES = 8
B_LOC = FULL_B // N_CORES  # 2 images per core
C1 = 0.01 ** 2
C2 = 0.03 ** 2
WS = 11
SIGMA = 1.5

# conv chunking: output chunks of 118 rows; input chunks of <=128 rows with 5-halo
CHUNK = 118
N_CH = 5  # ceil(512/118)
# per chunk: (input row start, input rows, output row start, output rows)
CH_IN0 = [0, 113, 231, 349, 467]
CH_INN = [123, 128, 128, 128, 45]
CH_OUT0 = [0, 118, 236, 354, 472]
CH_OUTN = [118, 118, 118, 118, 40]

BF16 = mybir.dt.bfloat16
F32 = mybir.dt.float32


def _gauss():
    """Gaussian taps, ULP-adjusted in bf16 so the bf16 window sums to 1.

    Raw bf16 rounding makes the window gain 0.99919, which biases every
    conv output by -0.08% and the final DSSIM by ~5e-3 relative. Nudging
    taps by +/-1 bf16 ULP (greedy, large taps first) recovers sum == 1
    exactly; measured end-to-end error drops to ~3.5e-4.
    """
    bf = ml_dtypes.bfloat16
    xs = np.arange(WS) - WS // 2
    g = np.exp(-(xs.astype(np.float64) ** 2) / (2.0 * SIGMA ** 2))
    g = (g / g.sum()).astype(np.float32)
    cand = g.astype(bf)
    for _ in range(4):
        for i in np.argsort(-g):
            base = cand.astype(np.float64).sum() - float(cand[i])
            u = np.array(cand[i], dtype=bf).view(np.uint16)
            opts = [
                np.array(u - 1, dtype=np.uint16).view(bf),
                cand[i],
                np.array(u + 1, dtype=np.uint16).view(bf),
            ]
            errs = [abs(base + float(o) - 1.0) for o in opts]
            cand[i] = opts[int(np.argmin(errs))]
    return cand.astype(np.float32)


def _g2(t, g):
    return g[t + 5] if abs(t) <= 5 else 0.0


def _band_mats():
    """Overlap-save band matrices, shared by pass1 (as rhs) and pass2 (as lhsT).

    mid  [128, 118]: M[j, i] = g(j - i - 5)   (input row = out_row - 5 + j)
    first[123, 118]: M[j, i] = g(j - i)       (rows clipped at image top)
    last [ 45,  40]: M[j, i] = g(j - i - 5)
    """
    g = _gauss()
    mid = np.zeros((128, 118), np.float32)
    for j in range(128):
        for i in range(118):
            mid[j, i] = _g2(j - i - 5, g)
    first = np.zeros((123, 118), np.float32)
    for j in range(123):
        for i in range(118):
            first[j, i] = _g2(j - i, g)
    last = np.zeros((45, 40), np.float32)
    for j in range(45):
        for i in range(40):
            last[j, i] = _g2(j - i - 5, g)
    return first, mid, last


def _act_recip(nc, out, in_):
    """activation(func=Reciprocal) without bass's precision guard."""
    eng = nc.scalar
    return eng.add_instruction(
        mybir.InstActivation(
            name=nc.get_next_instruction_name(),
            func=ACTF.Reciprocal,
            ins=[
                eng.lower_ap(in_),
                mybir.ImmediateValue(dtype=mybir.dt.float32, value=0.0),
                mybir.ImmediateValue(dtype=mybir.dt.float32, value=1.0),
                mybir.ImmediateValue(dtype=mybir.dt.float32, value=0.0),
            ],
            outs=[eng.lower_ap(out)],
        )
    )


def build_bass(n_sets=B_LOC * CH):
    nc = bacc.Bacc("TRN2", target_bir_lowering=False, debug=False)

    x_d = nc.dram_tensor("x", [B_LOC, CH, H, W], F32, kind="ExternalInput")
    y_d = nc.dram_tensor("y", [B_LOC, CH, H, W], F32, kind="ExternalInput")
    gf_d = nc.dram_tensor("gf", [123, 118], BF16, kind="ExternalInput")
    gm_d = nc.dram_tensor("gm", [128, 118], BF16, kind="ExternalInput")
    gl_d = nc.dram_tensor("gl", [45, 40], BF16, kind="ExternalInput")
    gfn_d = nc.dram_tensor("gfn", [123, 118], BF16, kind="ExternalInput")
    gmn_d = nc.dram_tensor("gmn", [128, 118], BF16, kind="ExternalInput")
    gln_d = nc.dram_tensor("gln", [45, 40], BF16, kind="ExternalInput")
    zf_d = nc.dram_tensor("zf", [83, W], F32, kind="ExternalInput")
    acc_d = nc.dram_tensor("acc", [128, 1], F32, kind="ExternalOutput")

    with tile.TileContext(nc) as tc:
        with (
            tc.tile_pool(name="consts", bufs=1) as consts,
            tc.tile_pool(name="inp", bufs=4) as inp,
            tc.tile_pool(name="prep", bufs=3) as prep,
            tc.tile_pool(name="t1", bufs=4) as t1p,
            tc.tile_pool(name="mapt", bufs=4) as mapt,
            tc.tile_pool(name="p1", bufs=2, space="PSUM") as p1p,
            tc.tile_pool(name="p2", bufs=2, space="PSUM") as p2p,
        ):
            gf = consts.tile([123, 118], BF16, tag="gf", name="gf")
            nc.sync.dma_start(out=gf, in_=gf_d[:, :])
            gm = consts.tile([128, 118], BF16, tag="gm", name="gm")
            nc.sync.dma_start(out=gm, in_=gm_d[:, :])
            gl = consts.tile([45, 40], BF16, tag="gl", name="gl")
            nc.sync.dma_start(out=gl, in_=gl_d[:, :])
            gfn = consts.tile([123, 118], BF16, tag="gfn", name="gfn")
            nc.sync.dma_start(out=gfn, in_=gfn_d[:, :])
            gmn = consts.tile([128, 118], BF16, tag="gmn", name="gmn")
            nc.sync.dma_start(out=gmn, in_=gmn_d[:, :])
            gln = consts.tile([45, 40], BF16, tag="gln", name="gln")
            nc.sync.dma_start(out=gln, in_=gln_d[:, :])

            def gpos(c):
                return (gf, gm, gl)[0 if c == 0 else (2 if c == N_CH - 1 else 1)]

            def gneg(c):
                return (gfn, gmn, gln)[0 if c == 0 else (2 if c == N_CH - 1 else 1)]

            acc = consts.tile([128, 1], F32, tag="acc", name="acc")
            nc.vector.memset(acc, 0.0)
            rsums = consts.tile([128, 32], F32, tag="rsums", name="rsums")
            nc.vector.memset(rsums, 0.0)
            iround = 0

            for iset in range(n_sets):
                b, c = divmod(iset, CH)
                if True:
                    # ---- load x, y in 5 overlapped row-chunks: [128, 5*512] f32
                    xo = inp.tile([128, N_CH * W], F32, tag="xo", name="xo")
                    yo = inp.tile([128, N_CH * W], F32, tag="yo", name="yo")
                    # zero the never-DMA'd halo rows of the edge chunks
                    # (tiny DMAs from a zeros constant; DMA engines are idle)
                    nc.sync.dma_start(out=xo[123:128, 0:W], in_=zf_d[0:5, :])
                    nc.sync.dma_start(out=yo[123:128, 0:W], in_=zf_d[0:5, :])
                    nc.sync.dma_start(out=xo[45:128, W * 4 : W * 5], in_=zf_d[0:83, :])
                    nc.sync.dma_start(out=yo[45:128, W * 4 : W * 5], in_=zf_d[0:83, :])
                    for k in range(N_CH):
                        r0, nr = CH_IN0[k], CH_INN[k]
                        nc.sync.dma_start(
                            out=xo[0:nr, W * k : W * k + W],
                            in_=x_d[b, c, r0 : r0 + nr, :],
                        )
                        nc.sync.dma_start(
                            out=yo[0:nr, W * k : W * k + W],
                            in_=y_d[b, c, r0 : r0 + nr, :],
                        )

                    # ---- prep: s, d on GPSIMD (f32 in, bf16 out); squares on
                    # GPSIMD too (set-level latency, hidden by input prefetch).
                    # First set runs on DVE in 512-col chunks so the pipeline
                    # fills fast instead of waiting ~10us for serial Pool ops.
                    st = prep.tile([128, N_CH * W], BF16, tag="s", name="s")
                    dt = prep.tile([128, N_CH * W], BF16, tag="d", name="d")
                    s2t = prep.tile([128, N_CH * W], BF16, tag="s2", name="s2")
                    d2t = prep.tile([128, N_CH * W], BF16, tag="d2", name="d2")
                    if iset == 0:
                        for k in range(N_CH):
                            sl = slice(W * k, W * k + W)
                            nc.vector.tensor_add(st[:, sl], xo[:, sl], yo[:, sl])
                            nc.vector.tensor_sub(dt[:, sl], xo[:, sl], yo[:, sl])
                            nc.vector.tensor_mul(s2t[:, sl], st[:, sl], st[:, sl])
                            nc.vector.tensor_mul(d2t[:, sl], dt[:, sl], dt[:, sl])
                    else:
                        nc.gpsimd.tensor_add(st, xo, yo)
                        nc.gpsimd.tensor_sub(dt, xo, yo)
                        nc.gpsimd.tensor_mul(s2t, st, st)
                        nc.gpsimd.tensor_mul(d2t, dt, dt)
                    srcs = (st, dt, s2t, d2t)

                    # ---- per 118-row w-chunk: pass1 (all 4 maps into a 4-bank
                    # psum tile), one batched evacuation, pass2, ssim map
                    for m in range(N_CH):
                        w0, pw = CH_IN0[m], CH_INN[m]
                        kin2, p2 = CH_INN[m], CH_OUTN[m]
                        lg, lgn = gpos(m), gneg(m)

                        t1c = t1p.tile([128, 4, W], BF16, tag="t1", name="t1c")
                        for half in range(2):
                            ps1 = p1p.tile([128, 2, W], F32, tag="p1", name="ps1")
                            for hi in range(2):
                                srcm = srcs[2 * half + hi]
                                for k in range(N_CH):
                                    kin = CH_INN[k]
                                    o0, on = CH_OUT0[k], CH_OUTN[k]
                                    nc.tensor.matmul(
                                        ps1[0:pw, hi, o0 : o0 + on],
                                        lhsT=srcm[
                                            0:kin, W * k + w0 : W * k + w0 + pw
                                        ],
                                        rhs=gpos(k)[0:kin, 0:on],
                                        start=(k == 0),
                                        stop=(k == N_CH - 1),
                                    )
                            dst = t1c[0:pw, 2 * half : 2 * half + 2, :]
                            if m in (1, 3):
                                nc.vector.tensor_copy(out=dst, in_=ps1[0:pw, :, :])
                            else:
                                nc.scalar.activation(
                                    out=dst, in_=ps1[0:pw, :, :], func=ACTF.Copy
                                )

                        psA = p2p.tile([118, 2, W], F32, tag="psAB", name="psA")
                        nc.tensor.matmul(
                            psA[0:p2, 0, :], lhsT=lg[0:kin2, 0:p2],
                            rhs=t1c[0:kin2, 0, :], start=True, stop=True,
                        )
                        nc.tensor.matmul(
                            psA[0:p2, 1, :], lhsT=lg[0:kin2, 0:p2],
                            rhs=t1c[0:kin2, 1, :], start=True, stop=True,
                        )
                        psB = p2p.tile([118, 2, W], F32, tag="psAB", name="psB")
                        nc.tensor.matmul(
                            psB[0:p2, 0, :], lhsT=lg[0:kin2, 0:p2],
                            rhs=t1c[0:kin2, 2, :], start=True, stop=False,
                        )
                        nc.tensor.matmul(
                            psB[0:p2, 0, :], lhsT=lgn[0:kin2, 0:p2],
                            rhs=t1c[0:kin2, 3, :], start=False, stop=True,
                        )
                        nc.tensor.matmul(
                            psB[0:p2, 1, :], lhsT=lg[0:kin2, 0:p2],
                            rhs=t1c[0:kin2, 2, :], start=True, stop=False,
                        )
                        nc.tensor.matmul(
                            psB[0:p2, 1, :], lhsT=lg[0:kin2, 0:p2],
                            rhs=t1c[0:kin2, 3, :], start=False, stop=True,
                        )

                        # map stage: ab = (S^2/2, D^2/2); wh = (w1/2+C2, w2/2+C2)
                        ab = mapt.tile([118, 2, W], BF16, tag="ab", name="ab")
                        nc.scalar.activation(
                            out=ab[0:p2, :, :], in_=psA[0:p2, :, :],
                            func=ACTF.Square, scale=float(np.sqrt(0.5)),
                        )
                        wh = mapt.tile([118, 2, W], BF16, tag="wh", name="wh")
                        nc.scalar.activation(
                            out=wh[0:p2, :, :], in_=psB[0:p2, :, :],
                            func=ACTF.Copy, scale=0.5, bias=C2,
                        )
                        uv = mapt.tile([118, 2, W], BF16, tag="uv", name="uv")
                        nc.vector.tensor_sub(
                            uv[0:p2, 0, :], ab[0:p2, 0, :], ab[0:p2, 1, :]
                        )
                        nc.vector.tensor_add(
                            uv[0:p2, 1, :], ab[0:p2, 0, :], ab[0:p2, 1, :]
                        )
                        nd = mapt.tile([118, 2, W], BF16, tag="nd", name="nd")
                        nc.vector.tensor_sub(
                            nd[0:p2, :, :], wh[0:p2, :, :], uv[0:p2, :, :]
                        )
                        numden = mapt.tile(
                            [118, 2, W], BF16, tag="numden", name="numden"
                        )
                        nc.vector.scalar_tensor_tensor(
                            out=numden[0:p2, :, :], in0=uv[0:p2, :, :], scalar=C1,
                            in1=nd[0:p2, :, :], op0=AOP.add, op1=AOP.mult,
                        )
                        rb = mapt.tile([118, W], BF16, tag="rb", name="rb")
                        _act_recip(nc, rb[0:p2, :], numden[0:p2, 1, :])
                        scr = mapt.tile([118, W], BF16, tag="scr", name="scr")
                        nc.vector.scalar_tensor_tensor(
                            out=scr[0:p2, :], in0=numden[0:p2, 0, :], scalar=1.0,
                            in1=rb[0:p2, :], op0=AOP.mult, op1=AOP.mult,
                            accum_out=rsums[0:p2, iround : iround + 1],
                        )
                        iround += 1

            nc.vector.tensor_reduce(
                out=acc, in_=rsums, op=AOP.add, axis=mybir.AxisListType.X
            )
            nc.sync.dma_start(out=acc_d[:, :], in_=acc)

    nc.finalize()
    return nc


_NC_CACHE = None


def kernel(x: np.ndarray, y: np.ndarray) -> np.ndarray:
    global _NC_CACHE
    if _NC_CACHE is None:
        _NC_CACHE = build_bass()
    nc = _NC_CACHE

    x = np.ascontiguousarray(np.asarray(x), dtype=np.float32)
    y = np.ascontiguousarray(np.asarray(y), dtype=np.float32)

    first, mid, last = _band_mats()
    bf = ml_dtypes.bfloat16
    consts = {
        "zf": np.zeros((83, W), np.float32),
        "gf": first.astype(bf),
        "gm": mid.astype(bf),
        "gl": last.astype(bf),
        "gfn": (-first).astype(bf),
        "gmn": (-mid).astype(bf),
        "gln": (-last).astype(bf),
    }

    in_maps = []
    for core in range(N_CORES):
        b0 = core * B_LOC
        in_maps.append(
            {
                "x": x[b0 : b0 + B_LOC],
                "y": y[b0 : b0 + B_LOC],
                **consts,
            }
        )

    res = run_bass_kernel_spmd(nc, in_maps, core_ids=list(range(N_CORES)))
    total = np.float64(0.0)
    for r in res.results:
        total += np.asarray(r["acc"], dtype=np.float64).sum()
    n_pix = FULL_B * CH * H * W
    return np.float32(1.0 - total / n_pix)


if __name__ == "__main__":
    rng = np.random.default_rng(0)
    x = rng.random((FULL_B, CH, H, W), dtype=np.float32)
    y = rng.random((FULL_B, CH, H, W), dtype=np.float32)
    print("kernel:", kernel(x, y))

